# revision 31
# baseline (speedup 1.0000x reference)
"""BERT self-attention layer (B=8, S=1024, H=12, Dh=64) on 8 trn2 NeuronCores.

Sharding: pure data-parallel over batch (1 batch item per core, weights
replicated).

Matmul inputs use bfloat16: on TRN2 the PE streams bf16 moving data at
1 col/cycle vs 2 cycles/col for fp32/f32r.  PSUM accumulation stays fp32
and the residual + LN path stays exact fp32 (final error ~1e-4 relative).

Structure (engine-balance driven):
  The softmax exp stream on ScalarE is the hard floor: H*S*S elements at
  1 elem/lane/cycle + 352-cycle per-instruction overhead ~= 110us.  So
  everything else hides under it:
  - phase A: xT via bf16 PE transposes, V projection, and the DVE
    StreamTranspose weight loads (permuted DMA spread over 4 queues).
  - phase B: per head h: [project QT/KTt for the pair when h is even]
    then a software-pipelined j-loop issuing scores(j+1) on the PE
    BEFORE ctx(j), so the PE never sits between ScalarE exps and the
    next head's scores.  One exp per (h, j): [128,1024] activation.
  - phase C: dense + residual + fused layernorm.
PSUM: sps 2x[128,1024] (4 banks) + cc [65,1024] (2) + qps [128,1024] (2).
"""

import os
import numpy as np
from contextlib import ExitStack

import concourse.bass as bass
import concourse.bacc as bacc
import concourse.tile as tile
from concourse import mybir
from concourse._compat import with_exitstack
from concourse.bass import ts, ds
from concourse.bass_utils import run_bass_kernel_spmd
from concourse.masks import make_identity

H = 12
DH = 64
D = 768
S = 1024
P = 128
KT_ = D // P  # 6 feature tiles
ST_ = S // P  # 8 sequence tiles
HB = DH + 1  # per-head V block width (64 value cols + ones col)
EPS = 1e-12
F32 = mybir.dt.float32
BF16 = mybir.dt.bfloat16
FT = mybir.ActivationFunctionType
ALU = mybir.AluOpType
N_CORES = 8
MDT = BF16  # dtype of tiles feeding matmuls

XT_BF16 = False  # bf16 PE transpose via bf16 PSUM returns garbage on HW
_debug_dump = None  # debug_hw.py sets this to a dict of DRAM APs
PROJ_UPFRONT = False  # bisect: hoist all QK projections before attention
FLAT_B = True  # bisect: flat (h,j) stream with lookahead vs round-2 pairs
ST_CAST = False  # bf16-out StreamTranspose fails the walrus ISA check


def _permuted_src(ap, col0, n_free_blocks):
    """DRAM AP enumerating src[32J+r, col0+c] for r,c in 32x32 blocks, in
    (r, J, c) order — the 32x32-block-permuted load feeding StreamTranspose."""
    rs = ap.ap[0][0]
    return bass.AP(
        tensor=ap.tensor,
        offset=ap.offset + col0,
        ap=[[rs, 32], [32 * rs, n_free_blocks], [1, 32]],
    )


def _load_T(nc, dest_pool, scratch_pool, src_ap, n_k_tiles, n_free, tag,
            dma_engines=None, cast_engine=None):
    """Transpose a [n_free, 128*n_k_tiles] DRAM matrix into n_k_tiles bf16
    sbuf tiles [128, n_free] without touching the PE:
    permuted DMA -> DVE 32x32 StreamTranspose -> cast to bf16.
    The whole matrix is fetched in just 4 DMAs (one per 32-partition
    group, 4-dim AP spanning all k tiles) — one InstDMACopy is split
    across all 16 SDMA engines, so fewer+bigger beats many small."""
    out = []
    if dma_engines is None:
        dma_engines = [nc.sync, nc.gpsimd, nc.scalar]
    ne = len(dma_engines)
    J = n_free // 32
    rs = src_ap.ap[0][0]
    perm = scratch_pool.tile([P, n_k_tiles * n_free], F32, tag="tsp", bufs=2,
                             name=f"{tag}perm")
    pv = perm.rearrange("(i r) (k j c) -> i r k j c", r=32, c=32, j=J)
    for i in range(4):
        src = bass.AP(
            tensor=src_ap.tensor,
            offset=src_ap.offset + 32 * i,
            ap=[[rs, 32], [P, n_k_tiles], [32 * rs, J], [1, 32]],
        )
        dma_engines[i % ne].dma_start(out=pv[i], in_=src)
    for kt in range(n_k_tiles):
        wt = dest_pool.tile([P, n_free], MDT, tag=tag, bufs=n_k_tiles,
                            name=f"{tag}_{src_ap.tensor.name}_{kt}")
        if ST_CAST:
            nc.vector.transpose(wt, perm[:, ds(kt * n_free, n_free)])
        else:
            tf = scratch_pool.tile([P, n_free], F32, tag="tst", bufs=2,
                                   name=f"{tag}t{kt}", padded_shape=[P, S])
            nc.vector.transpose(tf, perm[:, ds(kt * n_free, n_free)])
            ce = cast_engine or (nc.scalar if kt % 2 == 0 else nc.vector)
            if ce is nc.scalar:
                ce.copy(wt, tf)
            else:
                ce.tensor_copy(wt, tf)
        out.append(wt)
    return out


@with_exitstack
def bert_attn_kernel(
    ctx: ExitStack,
    tc: tile.TileContext,
    out_ap: bass.AP,
    x_ap: bass.AP,
    mask_ap: bass.AP,
    wq_ap: bass.AP,
    bq_ap: bass.AP,
    wk_ap: bass.AP,
    bk_ap: bass.AP,
    wv_ap: bass.AP,
    bv_ap: bass.AP,
    wd_ap: bass.AP,
    bd_ap: bass.AP,
    g_ap: bass.AP,
    b_ap: bass.AP,
    use_mask: bool,
    use_qkv_bias: bool,
    use_dense_bias: bool,
    use_ln_affine: bool,
):
    nc = tc.nc

    # ---- persistent pools ----
    const_pool = ctx.enter_context(tc.tile_pool(name="const", bufs=1))
    qkv_pool = ctx.enter_context(tc.tile_pool(name="qkv", bufs=1))
    ctxT_pool = ctx.enter_context(tc.tile_pool(name="ctxT", bufs=1))
    wT_pool = ctx.enter_context(tc.tile_pool(name="wT", bufs=1))
    tsc_pool = ctx.enter_context(tc.tile_pool(name="tsc", bufs=1))

    eps_t = const_pool.tile([P, 1], F32)
    nc.vector.memset(eps_t, EPS)
    ident = const_pool.tile([P, P], MDT if XT_BF16 else F32)
    make_identity(nc, ident)

    maskT = None
    if use_mask:
        maskT = const_pool.tile([P, ST_], F32)
        nc.sync.dma_start(out=maskT, in_=mask_ap.rearrange("(t p) -> p t", p=P))

    bq_t = bk_t = bv_bc = None
    if use_qkv_bias:
        bq_t = const_pool.tile([P, KT_], F32)
        nc.sync.dma_start(out=bq_t, in_=bq_ap.rearrange("(t p) -> p t", p=P))
        bk_t = const_pool.tile([P, KT_], F32)
        nc.sync.dma_start(out=bk_t, in_=bk_ap.rearrange("(t p) -> p t", p=P))
        bv_bc = const_pool.tile([P, D], F32)
        _bcast_load(nc, bv_bc, bv_ap, P)
    ones1 = bd_row = None
    if use_dense_bias:
        ones1 = const_pool.tile([1, P], MDT)
        nc.vector.memset(ones1, 1.0)
        bdf = const_pool.tile([1, D], F32)
        nc.sync.dma_start(out=bdf, in_=bd_ap[None, :])
        bd_row = const_pool.tile([1, D], MDT)
        nc.scalar.copy(bd_row, bdf)
    g_bc = b_bc = None
    if use_ln_affine:
        g_bc = const_pool.tile([P, D], F32)
        _bcast_load(nc, g_bc, g_ap, P)
        b_bc = const_pool.tile([P, D], F32)
        _bcast_load(nc, b_bc, b_ap, P)

    QT = [qkv_pool.tile([P, S], MDT, tag="QT", bufs=KT_, name=f"QT{i}")
          for i in range(KT_)]
    KTt = [qkv_pool.tile([P, S], MDT, tag="KTt", bufs=KT_, name=f"KTt{i}")
           for i in range(KT_)]
    vaug = [qkv_pool.tile([P, H * HB], MDT, tag="vaug", bufs=ST_,
                          name=f"vaug{i}") for i in range(ST_)]
    xn = [qkv_pool.tile([P, D], F32, tag="xn", bufs=ST_, name=f"xn{i}")
          for i in range(ST_)]

    # =========== phase A: x load/transpose, weight loads, V ===========
    with tc.tile_pool(name="xt", bufs=1) as xt_pool, \
         tc.tile_pool(name="ps_t", bufs=4, space="PSUM") as psum_t, \
         tc.tile_pool(name="ps_v", bufs=2, space="PSUM") as psum_v:

        for st in range(ST_):
            nc.sync.dma_start(out=xn[st], in_=x_ap[ts(st, P), :])
        # weight DMAs enqueue right behind the x rows on 4 queues
        wvT = _load_T(nc, wT_pool, tsc_pool, wv_ap, KT_, D, "wvT")
        wqT = _load_T(nc, wT_pool, tsc_pool, wq_ap, KT_, D, "wqT")
        wkT = _load_T(nc, wT_pool, tsc_pool, wk_ap, KT_, D, "wkT")

        xT = [qkv_pool.tile([P, S], MDT, tag="xT", bufs=KT_, name=f"xT{i}")
              for i in range(KT_)]
        if XT_BF16:
            xb = [xt_pool.tile([P, D], MDT, tag="xb", bufs=ST_, name=f"xb{i}")
                  for i in range(ST_)]
            for st in range(ST_):
                nc.scalar.copy(xb[st], xn[st])
            for st in range(ST_):
                for kt in range(KT_):
                    tps = psum_t.tile([P, P], MDT)
                    nc.tensor.transpose(tps, xb[st][:, ts(kt, P)], ident)
                    if (st + kt) % 2 == 0:
                        nc.scalar.copy(xT[kt][:, ts(st, P)], tps)
                    else:
                        nc.vector.tensor_copy(xT[kt][:, ts(st, P)], tps)
        else:
            for st in range(ST_):
                for kt in range(KT_):
                    tps = psum_t.tile([P, P], F32)
                    nc.tensor.transpose(tps, xn[st][:, ts(kt, P)], ident)
                    nc.scalar.copy(xT[kt][:, ts(st, P)], tps)

        # V = x @ Wv.T in natural layout, per-head [v0..v63, 1] blocks;
        # ones pre-filled, value cols overwritten by the evac.
        for v in vaug:
            nc.gpsimd.memset(v, 1.0)
        v3 = [v.rearrange("p (h c) -> p h c", c=HB) for v in vaug]
        for st in range(ST_):
            vps = psum_v.tile([P, S], F32, tag="vps", bufs=2,
                              name="vps")[:, 0:D]
            for kt in range(KT_):
                for c0 in range(0, D, 512):
                    cw = min(512, D - c0)
                    nc.tensor.matmul(
                        vps[:, ds(c0, cw)],
                        lhsT=xT[kt][:, ts(st, P)],
                        rhs=wvT[kt][:, ds(c0, cw)],
                        start=(kt == 0),
                        stop=(kt == KT_ - 1),
                    )
            vps3 = vps.rearrange("p (h c) -> p h c", c=DH)
            if use_qkv_bias:
                bv3 = bv_bc.rearrange("p (h c) -> p h c", c=DH)
                nc.vector.tensor_add(v3[st][:, :, 0:DH], vps3, bv3)
            elif st % 2 == 0:
                nc.vector.tensor_copy(v3[st][:, :, 0:DH], vps3)
            else:
                nc.scalar.copy(v3[st][:, :, 0:DH], vps3)

    ctxT = [ctxT_pool.tile([P, S], MDT, tag="ctxT", bufs=KT_, name=f"ctxT{i}")
            for i in range(KT_)]
    wdT = None

    # =========== phase B: projections interleaved with attention ===========
    # One flat software-pipelined (head, j) stream paced by the ScalarE exp
    # chain: scores(i+1) is issued to the PE before ctx(i) (so the next exp
    # input is always ready), QT/KTt projections for the NEXT head pair are
    # spread in 512-col chunks between steps of the odd head, and the cc
    # psum is released by a single fp32 copy to SBUF with the softmax
    # normalization running downstream, off the critical path.
    def project_chunk(psum_qp, wT_i, bias_t, dest, mt, qc, first, box=[None]):
        if first:
            box[0] = psum_qp.tile([P, S], F32, tag="qps", bufs=1, name="qps")
        qps = box[0]
        for kt in range(KT_):
            nc.tensor.matmul(
                qps[:, ds(qc, 512)],
                lhsT=wT_i[kt][:, ts(mt, P)],
                rhs=xT[kt][:, ds(qc, 512)],
                start=(kt == 0),
                stop=(kt == KT_ - 1),
            )
        if use_qkv_bias:
            nc.vector.tensor_scalar_add(dest[mt][:, ds(qc, 512)],
                                        qps[:, ds(qc, 512)],
                                        bias_t[:, mt : mt + 1])
        else:
            nc.vector.tensor_copy(dest[mt][:, ds(qc, 512)],
                                  qps[:, ds(qc, 512)])

    if PROJ_UPFRONT:
        with tc.tile_pool(name="ps_qp0", bufs=1, space="PSUM") as psum_qp0:
            for pn in range(KT_):
                for qc in range(0, S, 512):
                    project_chunk(psum_qp0, wqT, bq_t, QT, pn, qc, qc == 0)
                for qc in range(0, S, 512):
                    project_chunk(psum_qp0, wkT, bk_t, KTt, pn, qc, qc == 0)

    with tc.tile_pool(name="expT", bufs=1) as exp_pool, \
         tc.tile_pool(name="den", bufs=1) as den_pool, \
         tc.tile_pool(name="ps_qp", bufs=1, space="PSUM") as psum_qp, \
         tc.tile_pool(name="ps_s", bufs=2, space="PSUM") as psum_s, \
         tc.tile_pool(name="ps_cc", bufs=1, space="PSUM") as psum_cc:

        def scores(h, j):
            pr, hp = h // 2, DH * (h % 2)
            sps = psum_s.tile([P, S], F32, tag="sps", bufs=2, name="sps")
            for qc in range(0, S, 512):
                nc.tensor.matmul(
                    sps[:, ds(qc, 512)],
                    lhsT=KTt[pr][hp : hp + DH, ts(j, P)],
                    rhs=QT[pr][hp : hp + DH, ds(qc, 512)],
                    start=True,
                    stop=True,
                )
            return sps

        def normalize(h, cc_sb, den_sb):
            # den_sb is a base-partition-0 [1,S] copy of the denominator row:
            # the custom-DVE reciprocal misreads inputs at base_partition 64
            pr, hp = h // 2, DH * (h % 2)
            rec = den_pool.tile([1, S], F32, tag="rec", bufs=2)
            nc.vector.reciprocal_approx_fast(rec, den_sb)
            recb = den_pool.tile([DH, S], F32, tag="recb", bufs=2)
            nc.gpsimd.partition_broadcast(recb, rec)
            nc.vector.tensor_mul(ctxT[pr][hp : hp + DH, :],
                                 cc_sb[0:DH, :], recb)

        if not FLAT_B:
            # round-2-style pair loop (no lookahead): known-good reference
            for pr in range(H // 2):
                if pr == 1:
                    wdT = _load_T(nc, wT_pool, tsc_pool, wd_ap, KT_, D,
                                  "wdT", dma_engines=[nc.sync, nc.gpsimd],
                                  cast_engine=nc.vector)
                cc2 = [psum_cc.tile([HB, S], F32, tag="cc", bufs=2,
                                    name=f"cc{half}") for half in range(2)]
                for j in range(ST_):
                    sps2 = [psum_s.tile([P, S], F32, tag="sps", bufs=2,
                                        name=f"sps{half}")
                            for half in range(2)]
                    for qc in range(0, S, 512):
                        for half in range(2):
                            hp = DH * half
                            nc.tensor.matmul(
                                sps2[half][:, ds(qc, 512)],
                                lhsT=KTt[pr][hp : hp + DH, ts(j, P)],
                                rhs=QT[pr][hp : hp + DH, ds(qc, 512)],
                                start=True, stop=True,
                            )
                    ee = []
                    for half in range(2):
                        e = exp_pool.tile([P, S], MDT, tag="e", bufs=4,
                                          name=f"e{half}")
                        nc.scalar.activation(
                            e, sps2[half], FT.Exp,
                            bias=(maskT[:, j : j + 1] if use_mask else 0.0),
                            scale=0.125,
                        )
                        ee.append(e)
                    for half in range(2):
                        h2 = 2 * pr + half
                        for qc in range(0, S, 512):
                            nc.tensor.matmul(
                                cc2[half][:, ds(qc, 512)],
                                lhsT=vaug[j][:, ds(HB * h2, HB)],
                                rhs=ee[half][:, ds(qc, 512)],
                                start=(j == 0), stop=(j == ST_ - 1),
                            )
                for half in range(2):
                    h2 = 2 * pr + half
                    cc_sb = den_pool.tile([DH, S], F32, tag="cc_sb", bufs=2)
                    nc.vector.tensor_copy(cc_sb, cc2[half][0:DH, :])
                    den_sb = den_pool.tile([1, S], F32, tag="den_sb", bufs=2)
                    nc.vector.tensor_copy(den_sb, cc2[half][DH : DH + 1, :])
                    normalize(h2, cc_sb, den_sb)
        else:
            if not PROJ_UPFRONT:
                for qc in range(0, S, 512):
                    project_chunk(psum_qp, wqT, bq_t, QT, 0, qc, qc == 0)
                for qc in range(0, S, 512):
                    project_chunk(psum_qp, wkT, bk_t, KTt, 0, qc, qc == 0)
            steps = [(h, j) for h in range(H) for j in range(ST_)]
            sp = scores(0, 0)
            cc = None
            for i, (h, j) in enumerate(steps):
                if j == 0:
                    cc = psum_cc.tile([HB, S], F32, tag="cc", bufs=1,
                                      name="cc")
                sp_next = scores(*steps[i + 1]) if i + 1 < len(steps) else None
                e = exp_pool.tile([P, S], MDT, tag="e", bufs=3, name="e")
                nc.scalar.activation(
                    e, sp, FT.Exp,
                    bias=(maskT[:, j : j + 1] if use_mask else 0.0),
                    scale=0.125,
                )
                for qc in range(0, S, 512):
                    nc.tensor.matmul(
                        cc[:, ds(qc, 512)],
                        lhsT=vaug[j][:, ds(HB * h, HB)],
                        rhs=e[:, ds(qc, 512)],
                        start=(j == 0),
                        stop=(j == ST_ - 1),
                    )
                sp = sp_next
                if not PROJ_UPFRONT and h % 2 == 1 and h < H - 1:
                    pn = h // 2 + 1
                    if j < 2:
                        project_chunk(psum_qp, wqT, bq_t, QT, pn, 512 * j,
                                      j == 0)
                    elif j < 4:
                        project_chunk(psum_qp, wkT, bk_t, KTt, pn,
                                      512 * (j - 2), j == 2)
                if h == 1 and j == 5:
                    # overlap the Wd transpose-load with attention compute
                    wdT = _load_T(nc, wT_pool, tsc_pool, wd_ap, KT_, D,
                                  "wdT", dma_engines=[nc.sync, nc.gpsimd],
                                  cast_engine=nc.vector)
                if j == ST_ - 1:
                    # two evacs release the cc psum (den lands at partition
                    # 0); softmax normalization runs downstream on the copies
                    cc_sb = den_pool.tile([DH, S], F32, tag="cc_sb", bufs=2)
                    nc.vector.tensor_copy(cc_sb, cc[0:DH, :])
                    den_sb = den_pool.tile([1, S], F32, tag="den_sb", bufs=2)
                    nc.vector.tensor_copy(den_sb, cc[DH : DH + 1, :])
                    normalize(h, cc_sb, den_sb)

    # =========== phase C: dense + residual + layernorm ===========
    with tc.tile_pool(name="ln", bufs=2) as ln_pool, \
         tc.tile_pool(name="stat", bufs=4) as stat_pool, \
         tc.tile_pool(name="osb", bufs=3) as out_pool, \
         tc.tile_pool(name="ps_o", bufs=2, space="PSUM") as psum_o:

        for st in range(ST_):
            xr = xn[st]
            ops = psum_o.tile([P, D], F32, tag="ops", bufs=2)
            if use_dense_bias:
                for c0 in range(0, D, 512):
                    cw = min(512, D - c0)
                    nc.tensor.matmul(
                        ops[:, ds(c0, cw)], lhsT=ones1,
                        rhs=bd_row[:, ds(c0, cw)], start=True, stop=False,
                    )
            for kt in range(KT_):
                for c0 in range(0, D, 512):
                    cw = min(512, D - c0)
                    nc.tensor.matmul(
                        ops[:, ds(c0, cw)],
                        lhsT=ctxT[kt][:, ts(st, P)],
                        rhs=wdT[kt][:, ds(c0, cw)],
                        start=(kt == 0 and not use_dense_bias),
                        stop=(kt == KT_ - 1),
                    )
            # full = dense_out + x, accumulating the row-sum on the fly
            full = ln_pool.tile([P, D], F32, tag="full")
            sums = stat_pool.tile([P, 1], F32, tag="sums")
            nc.vector.scalar_tensor_tensor(
                out=full, in0=ops, scalar=1.0, in1=xr,
                op0=ALU.mult, op1=ALU.add, accum_out=sums,
            )
            # sum of squares on ScalarE; sq is a dead store
            sq = ln_pool.tile([P, D], F32, tag="sq")
            ssq = stat_pool.tile([P, 1], F32, tag="ssq")
            nc.scalar.activation(sq, full, FT.Square, accum_out=ssq)
            mu = stat_pool.tile([P, 1], F32, tag="mu")
            nc.vector.tensor_scalar_mul(mu, sums, 1.0 / D)
            mu2 = stat_pool.tile([P, 1], F32, tag="mu2")
            nc.vector.tensor_scalar_mul(mu2, mu, mu)
            var = stat_pool.tile([P, 1], F32, tag="var")
            nc.vector.scalar_tensor_tensor(
                out=var, in0=ssq, scalar=1.0 / D, in1=mu2,
                op0=ALU.mult, op1=ALU.subtract,
            )
            std = stat_pool.tile([P, 1], F32, tag="std")
            nc.scalar.activation(std, var, FT.Sqrt, bias=eps_t)
            rstd = stat_pool.tile([P, 1], F32, tag="rstd")
            nc.vector.reciprocal(rstd, std)
            osb = out_pool.tile([P, D], F32, tag="osb")
            nc.vector.tensor_scalar(
                out=osb, in0=full, scalar1=mu, scalar2=rstd,
                op0=ALU.subtract, op1=ALU.mult,
            )
            if use_ln_affine:
                nc.vector.tensor_mul(osb, osb, g_bc)
                nc.vector.tensor_add(osb, osb, b_bc)
            nc.sync.dma_start(out=out_ap[ts(st, P), :], in_=osb)

    if _debug_dump is not None:
        f32sc = ctx.enter_context(tc.tile_pool(name="dbgf32", bufs=1))
        for key, src in (("xT0", xT[0]), ("QT0", QT[0]), ("KT0", KTt[0]),
                         ("vaug0", vaug[0]), ("ctxT0", ctxT[0])):
            t = f32sc.tile(list(src.shape), F32, tag="dbg", bufs=5, name=key)
            nc.vector.tensor_copy(t, src)
            nc.sync.dma_start(out=_debug_dump[key], in_=t)


def _bcast_load(nc, out_tile, vec_ap, n_part):
    """DMA a [N] DRAM vector replicated across n_part partitions."""
    src = bass.AP(
        tensor=vec_ap.tensor,
        offset=vec_ap.offset,
        ap=[[0, n_part]] + [list(d) for d in vec_ap.ap],
    )
    nc.gpsimd.dma_start(out=out_tile, in_=src)


def build(flags):
    nc = bacc.Bacc(
        "TRN2", target_bir_lowering=False, debug=False, num_devices=N_CORES
    )
    aps = {}
    for name, shape in (
        ("hidden_states", [S, D]),
        ("attention_mask", [S]),
        ("Wq", [D, D]), ("bq", [D]),
        ("Wk", [D, D]), ("bk", [D]),
        ("Wv", [D, D]), ("bv", [D]),
        ("Wd", [D, D]), ("bd", [D]),
        ("ln_g", [D]), ("ln_b", [D]),
    ):
        aps[name] = nc.dram_tensor(name, shape, F32, kind="ExternalInput").ap()
    out = nc.dram_tensor("out", [S, D], F32, kind="ExternalOutput").ap()

    with tile.TileContext(nc) as tc:
        bert_attn_kernel(
            tc, out,
            aps["hidden_states"], aps["attention_mask"],
            aps["Wq"], aps["bq"], aps["Wk"], aps["bk"],
            aps["Wv"], aps["bv"], aps["Wd"], aps["bd"],
            aps["ln_g"], aps["ln_b"],
            *flags,
        )
    nc.compile()
    return nc


_CACHE = {}
last_results = None  # BassKernelResults of the most recent run (for test.py)


def kernel(**inputs):
    xs = {k: np.ascontiguousarray(np.asarray(v, dtype=np.float32))
          for k, v in inputs.items()}
    B = xs["hidden_states"].shape[0]
    assert B == N_CORES

    flags = (
        bool(np.any(xs["attention_mask"])),
        bool(np.any(xs["bq"]) or np.any(xs["bk"]) or np.any(xs["bv"])),
        bool(np.any(xs["bd"])),
        bool(np.any(xs["ln_g"] != 1.0) or np.any(xs["ln_b"])),
    )
    if flags not in _CACHE:
        _CACHE[flags] = build(flags)
    nc = _CACHE[flags]

    shared = {k: xs[k] for k in
              ("Wq", "bq", "Wk", "bk", "Wv", "bv", "Wd", "bd", "ln_g", "ln_b")}
    in_maps = [
        dict(
            hidden_states=xs["hidden_states"][i],
            attention_mask=np.ascontiguousarray(
                xs["attention_mask"][i].reshape(S)),
            **shared,
        )
        for i in range(N_CORES)
    ]
    trace = bool(int(os.environ.get("BERT_KERNEL_TRACE", "0")))
    res = run_bass_kernel_spmd(
        nc, in_maps, core_ids=list(range(N_CORES)), trace=trace
    )
    global last_results
    last_results = res
    return np.stack([res.results[i]["out"] for i in range(N_CORES)], axis=0)


if __name__ == "__main__":
    rng = np.random.default_rng(0)
    ins = {
        "hidden_states": rng.standard_normal((8, S, D), dtype=np.float32),
        "attention_mask": np.zeros((8, 1, 1, S), np.float32),
        "Wq": rng.standard_normal((D, D), dtype=np.float32) * 0.02,
        "bq": np.zeros(D, np.float32),
        "Wk": rng.standard_normal((D, D), dtype=np.float32) * 0.02,
        "bk": np.zeros(D, np.float32),
        "Wv": rng.standard_normal((D, D), dtype=np.float32) * 0.02,
        "bv": np.zeros(D, np.float32),
        "Wd": rng.standard_normal((D, D), dtype=np.float32) * 0.02,
        "bd": np.zeros(D, np.float32),
        "ln_g": np.ones(D, np.float32),
        "ln_b": np.zeros(D, np.float32),
    }
    out = kernel(**ins)
    print(out.shape, out.dtype, np.abs(out).max())
